# revision 8
# baseline (speedup 1.0000x reference)
"""Trainium2 Bass kernel for the label-selected log-softmax loss.

Math: per sample with logits [s, a] and label l in {0,1,2}:
    lp = log_softmax([s, a]);  err = (l==1)?lp[0] : (l==2)?lp[1] : 0
    loss = -mean(err)
With d = s - a:
    lp[0] = -softplus(-d),  lp[1] = -softplus(d)
    loss  = (1/B) * sum over l!=0 of softplus(c*d),  c = -1 if l==1, +1 if l==2
On device we compute sum over ALL samples of softplus(c*d) with c=0 for l==0
(which contributes softplus(0)=ln2 each); the host subtracts N0*ln2 in the
final unshard step.  Data parallel over 8 cores; each core reduces its shard
to a [128,1] per-partition partial that the host combines.
"""

import sys

sys.path.insert(0, "/opt/trn_rl_repo")

import numpy as np
import concourse.bass as bass
import concourse.bacc as bacc
import concourse.mybir as mybir
from concourse.tile import TileContext
from concourse.bass_utils import run_bass_kernel_spmd

N_CORES = 8
B = 8388608
NC = B // N_CORES  # 1048576 samples per core
P = 128
FTOT = NC // P  # 8192 free elements per partition
F = 2048  # tile free-dim
NT = FTOT // F

LN2 = float(np.log(2.0))

_cache = {}
last_result = None  # BassKernelResults of the most recent run (for profiling)


def _build():
    if "nc" in _cache:
        return _cache["nc"]
    nc = bacc.Bacc()
    sa_d = nc.declare_dram_parameter("sa", [P, 2 * FTOT], mybir.dt.float32, isOutput=False)
    c_d = nc.declare_dram_parameter("c", [P, FTOT], mybir.dt.int8, isOutput=False)
    out_d = nc.declare_dram_parameter("partial", [P, 1], mybir.dt.float32, isOutput=True)

    f32 = mybir.dt.float32
    CH = 2  # tiles per chunk; one wide Ln per chunk keeps ACT table reloads rare
    NCHUNK = NT // CH
    with TileContext(nc) as tc:
        with tc.tile_pool(name="io", bufs=4) as io, tc.tile_pool(name="zp", bufs=1) as zp:
            z_all = zp.tile([P, FTOT], f32, tag="z")
            acc = zp.tile([P, NCHUNK], f32, tag="acc")
            for ci in range(NCHUNK):
                for j in range(CH):
                    i = ci * CH + j
                    sa_t = io.tile([P, 2 * F], f32, tag="sa")
                    c_t = io.tile([P, F], mybir.dt.int8, tag="c")
                    nc.sync.dma_start(out=sa_t[:], in_=sa_d[:, i * 2 * F : (i + 1) * 2 * F])
                    nc.scalar.dma_start(out=c_t[:], in_=c_d[:, i * F : (i + 1) * F])
                    zi = z_all[:, i * F : (i + 1) * F]
                    nc.vector.tensor_sub(zi, sa_t[:, :F], sa_t[:, F : 2 * F])
                    nc.vector.tensor_mul(zi, zi, c_t[:])
                    # softplus(z) = ln(exp(z) + 1); Softplus itself is not in
                    # the compiler's ACT function tables, but exp+ln share one.
                    nc.scalar.activation(zi, zi, mybir.ActivationFunctionType.Exp)
                zc = z_all[:, ci * CH * F : (ci + 1) * CH * F]
                nc.scalar.activation(
                    zc,
                    zc,
                    mybir.ActivationFunctionType.Ln,
                    bias=1.0,
                    accum_out=acc[:, ci : ci + 1],
                )
            col = zp.tile([P, 1], f32, tag="col")
            nc.vector.reduce_sum(col[:], acc[:], axis=mybir.AxisListType.X)
            nc.sync.dma_start(out=out_d[:], in_=col[:])
    nc.compile()
    _cache["nc"] = nc
    return nc


def kernel(synonymy_score, antonymy_score, labels):
    global last_result
    s = np.ascontiguousarray(np.asarray(synonymy_score, dtype=np.float32).reshape(-1))
    a = np.ascontiguousarray(np.asarray(antonymy_score, dtype=np.float32).reshape(-1))
    lab = np.asarray(labels).reshape(-1)
    c = (lab == 2).astype(np.int8) - (lab == 1).astype(np.int8)
    n0 = int(np.count_nonzero(lab == 0))

    nc = _build()
    in_maps = []
    for k in range(N_CORES):
        sl = slice(k * NC, (k + 1) * NC)
        # Interleave s and a at tile granularity: tile i occupies columns
        # [2iF, 2(i+1)F) with the s-chunk first, then the a-chunk, so one DMA
        # feeds both operands of the subtract.
        sa = np.empty((P, 2 * FTOT), dtype=np.float32)
        sa3 = sa.reshape(P, FTOT // F, 2 * F)
        sa3[:, :, :F] = s[sl].reshape(P, FTOT // F, F)
        sa3[:, :, F:] = a[sl].reshape(P, FTOT // F, F)
        in_maps.append(
            {
                "sa": sa,
                "c": np.ascontiguousarray(c[sl]).reshape(P, FTOT),
            }
        )
    res = run_bass_kernel_spmd(nc, in_maps, list(range(N_CORES)))
    last_result = res
    total = 0.0
    for r in res.results:
        total += float(np.asarray(r["partial"], dtype=np.float64).sum())
    loss = (total - n0 * LN2) / B
    return np.float32(loss)


# revision 9
# speedup vs baseline: 1.2630x; 1.2630x over previous
"""Trainium2 Bass kernel for the label-selected log-softmax loss.

Math: per sample with logits [s, a] and label l in {0,1,2}:
    lp = log_softmax([s, a]);  err = (l==1)?lp[0] : (l==2)?lp[1] : 0
    loss = -mean(err)
With d = s - a:
    lp[0] = -softplus(-d) = -softplus(a-s),  lp[1] = -softplus(s-a)
so each selected sample contributes softplus(x-y) with (x,y) = (a,s) for
l==1 and (s,a) for l==2; l==0 samples contribute nothing.

Sharding strategy (data parallel over 8 cores): the host packs the selected
samples as (x,y) pairs — interleaved at tile granularity so one DMA feeds
both subtract operands — pads to a fixed per-core capacity with pairs whose
softplus underflows to exactly 0 (x=-30, y=30 -> softplus(-60) == 0 in f32),
and shards contiguously. Each core computes sum(softplus(x-y)) into a
[128,1] per-partition partial; the host sums partials / B.
"""

import sys

sys.path.insert(0, "/opt/trn_rl_repo")

import numpy as np
import concourse.bass as bass
import concourse.bacc as bacc
import concourse.mybir as mybir
from concourse.tile import TileContext
from concourse.bass_utils import run_bass_kernel_spmd

N_CORES = 8
B = 8388608
P = 128
F = 1024  # tile free-dim

_cache = {}
last_result = None  # BassKernelResults of the most recent run (for profiling)


def _build(ftot):
    """ftot: free elements per partition per core (capacity)."""
    if ftot in _cache:
        return _cache[ftot]
    nc = bacc.Bacc()
    sa_d = nc.declare_dram_parameter("sa", [P, 2 * ftot], mybir.dt.float32, isOutput=False)
    out_d = nc.declare_dram_parameter("partial", [P, 1], mybir.dt.float32, isOutput=True)

    f32 = mybir.dt.float32
    nt = ftot // F
    ch = 3 if nt % 3 == 0 else (2 if nt % 2 == 0 else 1)
    nchunk = nt // ch
    with TileContext(nc) as tc:
        with tc.tile_pool(name="io", bufs=6) as io, tc.tile_pool(name="zp", bufs=1) as zp:
            z_all = zp.tile([P, ftot], f32, tag="z")
            acc = zp.tile([P, nchunk], f32, tag="acc")
            for ci in range(nchunk):
                for j in range(ch):
                    i = ci * ch + j
                    sa_t = io.tile([P, 2 * F], f32, tag="sa")
                    nc.sync.dma_start(out=sa_t[:], in_=sa_d[:, i * 2 * F : (i + 1) * 2 * F])
                    zi = z_all[:, i * F : (i + 1) * F]
                    nc.vector.tensor_sub(zi, sa_t[:, :F], sa_t[:, F : 2 * F])
                    # softplus(z) = ln(exp(z) + 1); Softplus itself is not in
                    # the compiler's ACT function tables, but exp+ln share one.
                    nc.scalar.activation(zi, zi, mybir.ActivationFunctionType.Exp)
                zc = z_all[:, ci * ch * F : (ci + 1) * ch * F]
                nc.scalar.activation(
                    zc,
                    zc,
                    mybir.ActivationFunctionType.Ln,
                    bias=1.0,
                    accum_out=acc[:, ci : ci + 1],
                )
            col = zp.tile([P, 1], f32, tag="col")
            nc.vector.reduce_sum(col[:], acc[:], axis=mybir.AxisListType.X)
            nc.sync.dma_start(out=out_d[:], in_=col[:])
    nc.compile()
    _cache[ftot] = nc
    return nc


def kernel(synonymy_score, antonymy_score, labels):
    global last_result
    s = np.asarray(synonymy_score, dtype=np.float32).reshape(-1)
    a = np.asarray(antonymy_score, dtype=np.float32).reshape(-1)
    lab = np.asarray(labels).reshape(-1)

    swap = lab == 1
    keep = lab != 0
    x = np.where(swap, a, s)[keep]
    y = np.where(swap, s, a)[keep]
    n_sel = x.shape[0]

    # Fixed capacity: 6144 free elems/partition/core = 6.29M pairs total,
    # ~12% headroom over the expected 2/3 * B selected. Rebuild bigger if a
    # pathological label draw ever exceeds it.
    ftot = 6144
    while N_CORES * P * ftot < n_sel:
        ftot += 3072
    cap = N_CORES * P * ftot

    xp = np.full(cap, -30.0, dtype=np.float32)
    yp = np.full(cap, 30.0, dtype=np.float32)
    xp[:n_sel] = x
    yp[:n_sel] = y

    nc = _build(ftot)
    ncc = P * ftot  # pairs per core
    nt = ftot // F
    in_maps = []
    for k in range(N_CORES):
        sl = slice(k * ncc, (k + 1) * ncc)
        # Interleave x and y at tile granularity: tile i occupies columns
        # [2iF, 2(i+1)F) with the x-chunk first, then the y-chunk, so one DMA
        # feeds both operands of the subtract.
        sa = np.empty((P, 2 * ftot), dtype=np.float32)
        sa3 = sa.reshape(P, nt, 2 * F)
        sa3[:, :, :F] = xp[sl].reshape(P, nt, F)
        sa3[:, :, F:] = yp[sl].reshape(P, nt, F)
        in_maps.append({"sa": sa})
    res = run_bass_kernel_spmd(nc, in_maps, list(range(N_CORES)))
    last_result = res
    total = 0.0
    for r in res.results:
        total += float(np.asarray(r["partial"], dtype=np.float64).sum())
    return np.float32(total / B)


# revision 10
# speedup vs baseline: 1.3526x; 1.0710x over previous
"""Trainium2 Bass kernel for the label-selected log-softmax loss.

Math: per sample with logits [s, a] and label l in {0,1,2}:
    lp = log_softmax([s, a]);  err = (l==1)?lp[0] : (l==2)?lp[1] : 0
    loss = -mean(err)
With d = s - a:
    lp[0] = -softplus(-d) = -softplus(a-s),  lp[1] = -softplus(s-a)
so each selected sample contributes softplus(x-y) with (x,y) = (a,s) for
l==1 and (s,a) for l==2; l==0 samples contribute nothing.

Sharding strategy (data parallel over 8 cores): the host packs the selected
samples as (x,y) pairs — interleaved at tile granularity so one DMA feeds
both subtract operands — pads to a fixed per-core capacity with pairs whose
softplus underflows to exactly 0 (x=-30, y=30 -> softplus(-60) == 0 in f32),
and shards contiguously. Each core computes sum(softplus(x-y)) into a
[128,1] per-partition partial; the host sums partials / B.
"""

import sys

sys.path.insert(0, "/opt/trn_rl_repo")

import numpy as np
import ml_dtypes

_BF16 = np.dtype(ml_dtypes.bfloat16)

import concourse.bass as bass
import concourse.bacc as bacc
import concourse.mybir as mybir
from concourse.tile import TileContext
from concourse.bass_utils import run_bass_kernel_spmd

N_CORES = 8
B = 8388608
P = 128
F = 1024  # tile free-dim

_cache = {}
last_result = None  # BassKernelResults of the most recent run (for profiling)


def _build(ftot):
    """ftot: free elements per partition per core (capacity)."""
    if ftot in _cache:
        return _cache[ftot]
    nc = bacc.Bacc()
    sa_d = nc.declare_dram_parameter("sa", [P, 2 * ftot], mybir.dt.bfloat16, isOutput=False)
    out_d = nc.declare_dram_parameter("partial", [P, 1], mybir.dt.float32, isOutput=True)

    f32 = mybir.dt.float32
    nt = ftot // F
    ch = 3 if nt % 3 == 0 else (2 if nt % 2 == 0 else 1)
    nchunk = nt // ch
    with TileContext(nc) as tc:
        with tc.tile_pool(name="io", bufs=6) as io, tc.tile_pool(name="zp", bufs=1) as zp:
            z_all = zp.tile([P, ftot], f32, tag="z")
            acc = zp.tile([P, nchunk], f32, tag="acc")
            for ci in range(nchunk):
                for j in range(ch):
                    i = ci * ch + j
                    sa_t = io.tile([P, 2 * F], mybir.dt.bfloat16, tag="sa")
                    nc.sync.dma_start(out=sa_t[:], in_=sa_d[:, i * 2 * F : (i + 1) * 2 * F])
                    zi = z_all[:, i * F : (i + 1) * F]
                    nc.vector.tensor_sub(zi, sa_t[:, :F], sa_t[:, F : 2 * F])
                    # softplus(z) = ln(exp(z) + 1); Softplus itself is not in
                    # the compiler's ACT function tables, but exp+ln share one.
                    nc.scalar.activation(zi, zi, mybir.ActivationFunctionType.Exp)
                zc = z_all[:, ci * ch * F : (ci + 1) * ch * F]
                nc.scalar.activation(
                    zc,
                    zc,
                    mybir.ActivationFunctionType.Ln,
                    bias=1.0,
                    accum_out=acc[:, ci : ci + 1],
                )
            col = zp.tile([P, 1], f32, tag="col")
            nc.vector.reduce_sum(col[:], acc[:], axis=mybir.AxisListType.X)
            nc.sync.dma_start(out=out_d[:], in_=col[:])
    nc.compile()
    _cache[ftot] = nc
    return nc


def kernel(synonymy_score, antonymy_score, labels):
    global last_result
    s = np.asarray(synonymy_score, dtype=np.float32).reshape(-1)
    a = np.asarray(antonymy_score, dtype=np.float32).reshape(-1)
    lab = np.asarray(labels).reshape(-1)

    swap = lab == 1
    keep = lab != 0
    x = np.where(swap, a, s)[keep]
    y = np.where(swap, s, a)[keep]
    n_sel = x.shape[0]

    # Fixed capacity: 6144 free elems/partition/core = 6.29M pairs total,
    # ~12% headroom over the expected 2/3 * B selected. Rebuild bigger if a
    # pathological label draw ever exceeds it.
    ftot = 6144
    while N_CORES * P * ftot < n_sel:
        ftot += 3072
    cap = N_CORES * P * ftot

    xp = np.full(cap, -30.0, dtype=_BF16)
    yp = np.full(cap, 30.0, dtype=_BF16)
    xp[:n_sel] = x.astype(_BF16)
    yp[:n_sel] = y.astype(_BF16)

    nc = _build(ftot)
    ncc = P * ftot  # pairs per core
    nt = ftot // F
    in_maps = []
    for k in range(N_CORES):
        sl = slice(k * ncc, (k + 1) * ncc)
        # Interleave x and y at tile granularity: tile i occupies columns
        # [2iF, 2(i+1)F) with the x-chunk first, then the y-chunk, so one DMA
        # feeds both operands of the subtract.
        sa = np.empty((P, 2 * ftot), dtype=_BF16)
        sa3 = sa.reshape(P, nt, 2 * F)
        sa3[:, :, :F] = xp[sl].reshape(P, nt, F)
        sa3[:, :, F:] = yp[sl].reshape(P, nt, F)
        in_maps.append({"sa": sa})
    res = run_bass_kernel_spmd(nc, in_maps, list(range(N_CORES)))
    last_result = res
    total = 0.0
    for r in res.results:
        total += float(np.asarray(r["partial"], dtype=np.float64).sum())
    return np.float32(total / B)
